# revision 22
# baseline (speedup 1.0000x reference)
"""Trainium2 Bass kernel for GNN message passing:

    messages = e @ W_e.T + (h @ W_hu.T)[src] + (h @ W_hw.T)[tgt]

Strategy (8 NeuronCores, edge-parallel, bf16):
  - Edges are sharded 100k per core; h and the three weight matrices are
    replicated (bf16).
  - Phase 1 (per core): project the full node table once,
    hu = h @ W_hu.T and hw = h @ W_hw.T, into internal DRAM tables (bf16).
  - Phase 2 (per core): 4096-edge groups. ee = e @ W_e.T on the tensor
    engine into PSUM; hu[src] / hw[tgt] fetched with DMAGatherAnt
    (4096 rows x 256B per call); DVE sums psum + hu_g + hw_g into bf16;
    one DMA stores the group.
  - The gathers are spread over all 4 SWDGE queues: each queue's
    descriptor generation runs on a different GpSimd Q7 core pair, so 4
    gathers proceed concurrently (single-queue Q7 desc-gen rate is the
    bottleneck otherwise).  queue_num is rewritten post-scheduling to
    match each gather's Tile DMASW semaphore lane (a lane's sem may only
    be bumped from one queue).  num_swdge_queues=4 with a 32KB scratch
    gives 512-descriptor rings (a 4096-idx gather needs 257).
  - DMAGatherAnt indices are int16, so tables are addressed in two
    halves (split at row 25088).  The host permutes each core's edges
    into 4 buckets by (src-half, tgt-half), padded to fixed capacity so
    the SPMD program is identical on all cores.
  - DMA-descriptor coalescing: node j of each 512-node block is placed
    at hT column tau(j) so the phase-1 table store is 4 consecutive
    256B rows per partition (1KB descriptors).  The msgs store likewise
    writes DRAM row TPG*p+t from [partition p, tile t] (one contiguous
    run per partition per group); the host undoes both permutations
    when assembling the output.
"""
import os
from contextlib import ExitStack

import numpy as np
import ml_dtypes

import concourse.bass as bass
import concourse.tile as tile
from concourse import bacc, mybir
from concourse.bass_utils import run_bass_kernel_spmd

N_NODES = 50000
N_EDGES = 800000
IN_DIM = 128
OUT_DIM = 128
EDGE_DIM = 64
NCORES = 8

P = 128
NODES_PAD = 50176           # 98 * 512
HALF = 25088                # 49 * 512; int16 index range per half-table
HI_ROWS = NODES_PAD - HALF

EPC = N_EDGES // NCORES     # 100000 edges per core
G_EDGES = 4096              # edges per gather group (257 ring descs/lane; 512 max)
TPG = G_EDGES // P          # msgs tiles per group
# Bucket capacities in groups, bucket = 2*(src>=HALF)+(tgt>=HALF).
# Worst per-core counts for the jax key-0 input: ~[25217, 25215, 25140, 25128].
CAPS = [7, 7, 7, 7]
NGRP = sum(CAPS)            # 28 groups
EPC_PAD = NGRP * G_EDGES    # 114688
SEG_EDGE_START = [0]
for _c in CAPS:
    SEG_EDGE_START.append(SEG_EDGE_START[-1] + _c * G_EDGES)

F32 = mybir.dt.float32
BF16 = mybir.dt.bfloat16
I16 = mybir.dt.int16
NPBF16 = ml_dtypes.bfloat16

_CACHE = {}
LAST = {}


def _build():
    nc = bacc.Bacc(
        "TRN2",
        target_bir_lowering=False,
        debug=False,
        enable_asserts=True,
        num_devices=NCORES,
        num_swdge_queues=4,
        dynamic_dma_scratch_size=32768,
    )

    hT = nc.dram_tensor("hT", [P, NODES_PAD], BF16, kind="ExternalInput").ap()
    Wcat = nc.dram_tensor("Wcat", [P, 2 * OUT_DIM], BF16, kind="ExternalInput").ap()
    WeT = nc.dram_tensor("WeT", [2 * EDGE_DIM, OUT_DIM], BF16, kind="ExternalInput").ap()
    eP = nc.dram_tensor("eP", [NGRP, P, G_EDGES // 2], BF16, kind="ExternalInput").ap()
    sidx = nc.dram_tensor("sidx", [P, NGRP * (G_EDGES // 16)], I16, kind="ExternalInput").ap()
    tidx = nc.dram_tensor("tidx", [P, NGRP * (G_EDGES // 16)], I16, kind="ExternalInput").ap()
    msgs = nc.dram_tensor("msgs", [EPC_PAD, OUT_DIM], BF16, kind="ExternalOutput").ap()

    # lo/hi are separate tensors so phase-2 lo-lo gathers only depend on the
    # lo-half table writes and can overlap the rest of phase 1.
    hu_lo = nc.dram_tensor("hu_lo", [HALF, OUT_DIM], BF16).ap()
    hu_hi = nc.dram_tensor("hu_hi", [HI_ROWS, OUT_DIM], BF16).ap()
    hw_lo = nc.dram_tensor("hw_lo", [HALF, OUT_DIM], BF16).ap()
    hw_hi = nc.dram_tensor("hw_hi", [HI_ROWS, OUT_DIM], BF16).ap()

    with tile.TileContext(nc) as tc:
        with ExitStack() as ctx:
            wpool = ctx.enter_context(tc.tile_pool(name="w", bufs=1))
            idxpool = ctx.enter_context(tc.tile_pool(name="idx", bufs=1))
            hpool = ctx.enter_context(tc.tile_pool(name="h", bufs=3))
            p1psum = ctx.enter_context(tc.tile_pool(name="p1psum", bufs=4, space="PSUM"))
            opool = ctx.enter_context(tc.tile_pool(name="o", bufs=3))
            epool = ctx.enter_context(tc.tile_pool(name="e", bufs=4))
            p2psum = ctx.enter_context(tc.tile_pool(name="p2psum", bufs=2, space="PSUM"))
            gpool = ctx.enter_context(tc.tile_pool(name="g", bufs=6))
            mpool = ctx.enter_context(tc.tile_pool(name="m", bufs=3))

            wcat_t = wpool.tile([P, 2 * OUT_DIM], BF16)
            nc.scalar.dma_start(out=wcat_t[:], in_=Wcat[:])
            wet_t = wpool.tile([2 * EDGE_DIM, OUT_DIM], BF16)
            nc.scalar.dma_start(out=wet_t[:], in_=WeT[:])
            sidx_t = idxpool.tile([P, NGRP * (G_EDGES // 16)], I16)
            nc.sync.dma_start(out=sidx_t[:], in_=sidx[:])
            tidx_t = idxpool.tile([P, NGRP * (G_EDGES // 16)], I16)
            nc.sync.dma_start(out=tidx_t[:], in_=tidx[:])

            # ---- Phase 1 blocks: hu/hw node tables ------------------------
            # 512 nodes per block: 4 matmuls through 2-bank PSUM tiles, two
            # wide DVE copies (fp32 psum -> bf16), then one batched store per
            # table.  Node tau-permutation (see module docstring) makes each
            # partition's 4 rows consecutive in DRAM -> 1KB descriptors.
            def p1_block(i):
                hb = hpool.tile([P, 512], BF16)
                nc.scalar.dma_start(out=hb[:], in_=hT[:, i * 512 : (i + 1) * 512])
                ot = opool.tile([P, 1024], BF16)
                for half in range(2):
                    ps = p1psum.tile([P, 512], F32)
                    for s in range(2):
                        nc.tensor.matmul(
                            out=ps[:, s * 256 : (s + 1) * 256],
                            lhsT=hb[:, (half * 2 + s) * P : (half * 2 + s + 1) * P],
                            rhs=wcat_t[:],
                            start=True,
                            stop=True,
                        )
                    nc.vector.tensor_copy(out=ot[:, half * 512 : (half + 1) * 512], in_=ps[:])
                ot3 = ot[:].rearrange("p (s x) -> p s x", s=4)
                if i < HALF // 512:
                    hu_dst, hw_dst, n0 = hu_lo, hw_lo, i * 512
                else:
                    hu_dst, hw_dst, n0 = hu_hi, hw_hi, i * 512 - HALF
                nc.sync.dma_start(
                    out=hu_dst[n0 : n0 + 512, :].rearrange("(p s) d -> p s d", p=P),
                    in_=ot3[:, :, 0:OUT_DIM],
                )
                nc.scalar.dma_start(
                    out=hw_dst[n0 : n0 + 512, :].rearrange("(p s) d -> p s d", p=P),
                    in_=ot3[:, :, OUT_DIM : 2 * OUT_DIM],
                )

            # ---- Phase 2 groups: per-edge messages ------------------------
            def p2_group(G):
                seg = 0
                while G >= SEG_EDGE_START[seg + 1] // G_EDGES:
                    seg += 1
                hu_src = (hu_lo if seg < 2 else hu_hi)[:]
                hw_src = (hw_lo if seg % 2 == 0 else hw_hi)[:]
                eb = epool.tile([P, G_EDGES // 2], BF16)
                nc.scalar.dma_start(out=eb[:], in_=eP[G])

                hu_g = gpool.tile([P, G_EDGES], BF16, tag="hu_g")
                nc.gpsimd.dma_gather(
                    out_ap=hu_g[:].rearrange("p (c d) -> p c d", c=TPG),
                    in_ap=hu_src,
                    idxs_ap=sidx_t[:, G * (G_EDGES // 16) : (G + 1) * (G_EDGES // 16)],
                    num_idxs=G_EDGES,
                    num_idxs_reg=G_EDGES,
                    elem_size=OUT_DIM,
                    single_packet=False,
                    queue_num=(2 * G) % 4,
                )
                hw_g = gpool.tile([P, G_EDGES], BF16, tag="hw_g")
                nc.gpsimd.dma_gather(
                    out_ap=hw_g[:].rearrange("p (c d) -> p c d", c=TPG),
                    in_ap=hw_src,
                    idxs_ap=tidx_t[:, G * (G_EDGES // 16) : (G + 1) * (G_EDGES // 16)],
                    num_idxs=G_EDGES,
                    num_idxs_reg=G_EDGES,
                    elem_size=OUT_DIM,
                    single_packet=False,
                    queue_num=(2 * G + 1) % 4,
                )

                mt = mpool.tile([P, G_EDGES], BF16)
                half_tiles = TPG // 2
                for hhalf in range(G_EDGES // 1024):
                    ps = p2psum.tile([P, 1024], F32)
                    for q in range(8):
                        t = hhalf * 8 + q
                        pb = 0 if t < half_tiles else EDGE_DIM
                        col = t % half_tiles
                        nc.tensor.matmul(
                            out=ps[:, q * P : (q + 1) * P],
                            lhsT=eb[pb : pb + EDGE_DIM, col * P : (col + 1) * P],
                            rhs=wet_t[pb : pb + EDGE_DIM, :],
                            start=True,
                            stop=True,
                        )
                    sl = slice(hhalf * 1024, (hhalf + 1) * 1024)
                    nc.vector.tensor_add(out=mt[:, sl], in0=ps[:], in1=hu_g[:, sl])
                    nc.vector.tensor_add(out=mt[:, sl], in0=mt[:, sl], in1=hw_g[:, sl])

                nc.sync.dma_start(
                    out=msgs[G * G_EDGES : (G + 1) * G_EDGES, :].rearrange(
                        "(p t) d -> p t d", p=P
                    ),
                    in_=mt[:].rearrange("p (t d) -> p t d", t=TPG),
                )

            # Emission order: lo table blocks first, then the (lo,lo) edge
            # segment (its gathers only need the lo tables), then the hi
            # blocks, then the remaining segments.
            for i in range(HALF // 512):
                p1_block(i)
            for G in range(CAPS[0]):
                p2_group(G)
            for i in range(HALF // 512, NODES_PAD // 512):
                p1_block(i)
            for G in range(CAPS[0], NGRP):
                p2_group(G)

    # Tile assigns each SWDGE DMA a DMASW completion-sem lane in *scheduled*
    # order, and a lane's sem may only ever be bumped from one SWDGE queue.
    # The scheduler reorders gathers, so a static queue rotation can put two
    # queues on one lane.  Rewrite queue_num = lane % 4 after scheduling.
    from concourse.tile_scheduler import PROC_NAME_TO_IDX

    sw0 = PROC_NAME_TO_IDX["DMASW0"]
    for inst in nc.inst_map.values():
        if isinstance(inst, mybir.InstDMAGatherAnt):
            lane = inst.bass_scheduled_proc - sw0
            assert 0 <= lane < 8, (inst.name, inst.bass_scheduled_proc)
            inst.queue_num = lane % 4

    nc.compile()
    return nc


def get_nc():
    if "nc" not in _CACHE:
        _CACHE["nc"] = _build()
    return _CACHE["nc"]


# tau: hT column j holds node tau(j), so that the phase-1 store
# "(p s) d -> p s d" (table row 4p+s <- psum partition p, slice s) lands
# node n at table row n.  psum partition p / slice s corresponds to hT
# column 512i + 128s + p, and the store writes it to table row 512i+4p+s.
def _tau(j):
    blk, off = j // 512, j % 512
    s, p = off // 128, off % 128
    return blk * 512 + 4 * p + s


_TAU = _tau(np.arange(NODES_PAD))


def _prep_in_maps(h, e, edge_index, W_e, W_hu, W_hw):
    """Returns (in_maps, pos_list): pos_list[c][i] = row of core c's device
    output holding original edge c*EPC+i."""
    h = np.ascontiguousarray(np.asarray(h, dtype=np.float32))
    e = np.ascontiguousarray(np.asarray(e, dtype=np.float32))
    src = np.asarray(edge_index[0]).astype(np.int64)
    tgt = np.asarray(edge_index[1]).astype(np.int64)

    hpad = np.zeros((NODES_PAD, IN_DIM), dtype=np.float32)
    hpad[:N_NODES] = h
    hT = np.ascontiguousarray(hpad[_TAU].T.astype(NPBF16))
    Wcat = np.ascontiguousarray(
        np.concatenate([np.asarray(W_hu, np.float32).T, np.asarray(W_hw, np.float32).T], axis=1)
    ).astype(NPBF16)
    # stacked twice so phase 2 has a copy at SBUF base partition 0 and 64
    WeT_np = np.asarray(W_e, np.float32).T
    WeT = np.ascontiguousarray(np.vstack([WeT_np, WeT_np])).astype(NPBF16)

    in_maps = []
    pos_list = []
    for c in range(NCORES):
        sl = slice(c * EPC, (c + 1) * EPC)
        sc, tc_, ec = src[sl], tgt[sl], e[sl]
        bucket = 2 * (sc >= HALF).astype(np.int64) + (tc_ >= HALF).astype(np.int64)

        e_pad = np.zeros((EPC_PAD, EDGE_DIM), dtype=np.float32)
        s16 = np.zeros((EPC_PAD,), dtype=np.int16)
        t16 = np.zeros((EPC_PAD,), dtype=np.int16)
        pos = np.empty((EPC,), dtype=np.int64)
        for b in range(4):
            selb = np.flatnonzero(bucket == b)
            if len(selb) > CAPS[b] * G_EDGES:
                raise RuntimeError(
                    f"bucket {b} overflow on core {c}: {len(selb)} > {CAPS[b] * G_EDGES}"
                )
            base = SEG_EDGE_START[b]
            pos[selb] = base + np.arange(len(selb))
            e_pad[base : base + len(selb)] = ec[selb]
            s16[base : base + len(selb)] = (sc[selb] - HALF * (b >> 1)).astype(np.int16)
            t16[base : base + len(selb)] = (tc_[selb] - HALF * (b & 1)).astype(np.int16)

        # device msgs row for padded slot j: group g = j // G_EDGES,
        # within-group slot sl sits at psum partition p = sl % 128, tile
        # t = sl // 128, stored at DRAM row g*G_EDGES + TPG*p + t.
        g_, sl_ = pos // G_EDGES, pos % G_EDGES
        pos = g_ * G_EDGES + TPG * (sl_ % P) + sl_ // P

        ePc = np.ascontiguousarray(
            e_pad.reshape(NGRP, 2, G_EDGES // 2, EDGE_DIM).transpose(0, 1, 3, 2)
        ).reshape(NGRP, P, G_EDGES // 2).astype(NPBF16)

        # dma_gather index layout: value j of group g sits at
        # [j % 16, g*(G/16) + j//16], replicated across the 8 gpsimd cores.
        def idx_layout(v16):
            a16 = v16.reshape(NGRP, G_EDGES // 16, 16).transpose(2, 0, 1).reshape(
                16, NGRP * (G_EDGES // 16)
            )
            return np.ascontiguousarray(np.tile(a16, (8, 1)))

        in_maps.append(
            {
                "hT": hT,
                "Wcat": Wcat,
                "WeT": WeT,
                "eP": ePc,
                "sidx": idx_layout(s16),
                "tidx": idx_layout(t16),
            }
        )
        pos_list.append(pos)
    return in_maps, pos_list


def _install_ntff_hook():
    """Best-effort: register the axon NTFF profile hook when the image's
    antenv package lacks axon_hooks (needed only for trace=True runs)."""
    import sys
    import types

    try:
        from antenv.axon_hooks import get_axon_ntff_profile_hook  # noqa: F401

        return
    except ImportError:
        pass
    try:
        from trn_agent_boot.trn_boot import _ntff_profile_via_ctypes

        hook = _ntff_profile_via_ctypes("/opt/axon/libaxon_pjrt.so")
        mod = types.ModuleType("antenv.axon_hooks")
        mod._hook = hook
        mod.get_axon_ntff_profile_hook = lambda: mod._hook
        mod.set_axon_ntff_profile_hook = lambda h: setattr(mod, "_hook", h)
        sys.modules["antenv.axon_hooks"] = mod
        import antenv

        antenv.axon_hooks = mod
    except Exception:
        pass


def kernel(h, e, edge_index, W_e, W_hu, W_hw):
    nc = get_nc()
    in_maps, pos_list = _prep_in_maps(h, e, edge_index, W_e, W_hu, W_hw)
    trace = bool(int(os.environ.get("KERNEL_TRACE", "0")))
    if trace:
        _install_ntff_hook()
    res = run_bass_kernel_spmd(nc, in_maps, list(range(NCORES)), trace=trace)
    LAST["exec_time_ns"] = res.exec_time_ns
    LAST["results"] = res
    out = np.empty((N_EDGES, OUT_DIM), dtype=np.float32)
    for c in range(NCORES):
        m = np.asarray(res.results[c]["msgs"])
        out[c * EPC : (c + 1) * EPC] = m[pos_list[c]].astype(np.float32)
    return out


# revision 24
# speedup vs baseline: 1.0963x; 1.0963x over previous
"""Trainium2 Bass kernel for GNN message passing:

    messages = e @ W_e.T + (h @ W_hu.T)[src] + (h @ W_hw.T)[tgt]

Strategy (8 NeuronCores, edge-parallel, bf16):
  - Edges are sharded 100k per core; h and the three weight matrices are
    replicated (bf16).
  - Phase 1 (per core): project the full node table once,
    hu = h @ W_hu.T and hw = h @ W_hw.T, into internal DRAM tables (bf16).
  - Phase 2 (per core): 4096-edge groups. ee = e @ W_e.T on the tensor
    engine into PSUM; hu[src] / hw[tgt] fetched with DMAGatherAnt
    (4096 rows x 256B per call); DVE sums psum + hu_g + hw_g into bf16;
    one DMA stores the group.
  - The gathers are spread over all 4 SWDGE queues: each queue's
    descriptor generation runs on a different GpSimd Q7 core pair, so 4
    gathers proceed concurrently (single-queue Q7 desc-gen rate is the
    bottleneck otherwise).  queue_num is rewritten post-scheduling to
    match each gather's Tile DMASW semaphore lane (a lane's sem may only
    be bumped from one queue).  num_swdge_queues=4 with a 32KB scratch
    gives 512-descriptor rings (a 4096-idx gather needs 257).
  - DMAGatherAnt indices are int16, so tables are addressed in two
    halves (split at row 25088).  The host permutes each core's edges
    into 4 buckets by (src-half, tgt-half), padded to fixed capacity so
    the SPMD program is identical on all cores.
  - DMA-descriptor coalescing: node j of each 512-node block is placed
    at hT column tau(j) so the phase-1 table store is 4 consecutive
    256B rows per partition (1KB descriptors).  The msgs store likewise
    writes DRAM row TPG*p+t from [partition p, tile t] (one contiguous
    run per partition per group); the host undoes both permutations
    when assembling the output.
"""
import os
from contextlib import ExitStack

import numpy as np
import ml_dtypes

import concourse.bass as bass
import concourse.tile as tile
from concourse import bacc, mybir
from concourse.bass_utils import run_bass_kernel_spmd

N_NODES = 50000
N_EDGES = 800000
IN_DIM = 128
OUT_DIM = 128
EDGE_DIM = 64
NCORES = 8

P = 128
NODES_PAD = 50176           # 98 * 512
HALF = 25088                # 49 * 512; int16 index range per half-table
HI_ROWS = NODES_PAD - HALF

EPC = N_EDGES // NCORES     # 100000 edges per core
G_EDGES = 2048              # edges per gather group (129 ring descs/lane -> two
                            # gathers pipeline per 512-desc queue ring)
TPG = G_EDGES // P          # msgs tiles per group
# Bucket capacities in groups, bucket = 2*(src>=HALF)+(tgt>=HALF).
# Worst per-core counts for the jax key-0 input: ~[25217, 25215, 25140, 25128].
CAPS = [13, 13, 13, 13]
NGRP = sum(CAPS)            # 52 groups
EPC_PAD = NGRP * G_EDGES    # 106496
SEG_EDGE_START = [0]
for _c in CAPS:
    SEG_EDGE_START.append(SEG_EDGE_START[-1] + _c * G_EDGES)

F32 = mybir.dt.float32
BF16 = mybir.dt.bfloat16
I16 = mybir.dt.int16
NPBF16 = ml_dtypes.bfloat16

_CACHE = {}
LAST = {}


def _build():
    nc = bacc.Bacc(
        "TRN2",
        target_bir_lowering=False,
        debug=False,
        enable_asserts=True,
        num_devices=NCORES,
        num_swdge_queues=4,
        dynamic_dma_scratch_size=32768,
    )

    hT = nc.dram_tensor("hT", [P, NODES_PAD], BF16, kind="ExternalInput").ap()
    Wcat = nc.dram_tensor("Wcat", [P, 2 * OUT_DIM], BF16, kind="ExternalInput").ap()
    WeT = nc.dram_tensor("WeT", [2 * EDGE_DIM, OUT_DIM], BF16, kind="ExternalInput").ap()
    eP = nc.dram_tensor("eP", [NGRP, P, G_EDGES // 2], BF16, kind="ExternalInput").ap()
    sidx = nc.dram_tensor("sidx", [P, NGRP * (G_EDGES // 16)], I16, kind="ExternalInput").ap()
    tidx = nc.dram_tensor("tidx", [P, NGRP * (G_EDGES // 16)], I16, kind="ExternalInput").ap()
    msgs = nc.dram_tensor("msgs", [EPC_PAD, OUT_DIM], BF16, kind="ExternalOutput").ap()

    # lo/hi are separate tensors so phase-2 lo-lo gathers only depend on the
    # lo-half table writes and can overlap the rest of phase 1.
    hu_lo = nc.dram_tensor("hu_lo", [HALF, OUT_DIM], BF16).ap()
    hu_hi = nc.dram_tensor("hu_hi", [HI_ROWS, OUT_DIM], BF16).ap()
    hw_lo = nc.dram_tensor("hw_lo", [HALF, OUT_DIM], BF16).ap()
    hw_hi = nc.dram_tensor("hw_hi", [HI_ROWS, OUT_DIM], BF16).ap()

    with tile.TileContext(nc) as tc:
        with ExitStack() as ctx:
            wpool = ctx.enter_context(tc.tile_pool(name="w", bufs=1))
            idxpool = ctx.enter_context(tc.tile_pool(name="idx", bufs=1))
            hpool = ctx.enter_context(tc.tile_pool(name="h", bufs=3))
            p1psum = ctx.enter_context(tc.tile_pool(name="p1psum", bufs=4, space="PSUM"))
            opool = ctx.enter_context(tc.tile_pool(name="o", bufs=3))
            epool = ctx.enter_context(tc.tile_pool(name="e", bufs=6))
            p2psum = ctx.enter_context(tc.tile_pool(name="p2psum", bufs=2, space="PSUM"))
            gpool = ctx.enter_context(tc.tile_pool(name="g", bufs=12))
            mpool = ctx.enter_context(tc.tile_pool(name="m", bufs=6))

            wcat_t = wpool.tile([P, 2 * OUT_DIM], BF16)
            nc.scalar.dma_start(out=wcat_t[:], in_=Wcat[:])
            wet_t = wpool.tile([2 * EDGE_DIM, OUT_DIM], BF16)
            nc.scalar.dma_start(out=wet_t[:], in_=WeT[:])
            sidx_t = idxpool.tile([P, NGRP * (G_EDGES // 16)], I16)
            nc.sync.dma_start(out=sidx_t[:], in_=sidx[:])
            tidx_t = idxpool.tile([P, NGRP * (G_EDGES // 16)], I16)
            nc.sync.dma_start(out=tidx_t[:], in_=tidx[:])

            # ---- Phase 1 blocks: hu/hw node tables ------------------------
            # 512 nodes per block: 4 matmuls through 2-bank PSUM tiles, two
            # wide DVE copies (fp32 psum -> bf16), then one batched store per
            # table.  Node tau-permutation (see module docstring) makes each
            # partition's 4 rows consecutive in DRAM -> 1KB descriptors.
            def p1_block(i):
                hb = hpool.tile([P, 512], BF16)
                nc.scalar.dma_start(out=hb[:], in_=hT[:, i * 512 : (i + 1) * 512])
                ot = opool.tile([P, 1024], BF16)
                for half in range(2):
                    ps = p1psum.tile([P, 512], F32)
                    for s in range(2):
                        nc.tensor.matmul(
                            out=ps[:, s * 256 : (s + 1) * 256],
                            lhsT=hb[:, (half * 2 + s) * P : (half * 2 + s + 1) * P],
                            rhs=wcat_t[:],
                            start=True,
                            stop=True,
                        )
                    nc.vector.tensor_copy(out=ot[:, half * 512 : (half + 1) * 512], in_=ps[:])
                ot3 = ot[:].rearrange("p (s x) -> p s x", s=4)
                if i < HALF // 512:
                    hu_dst, hw_dst, n0 = hu_lo, hw_lo, i * 512
                else:
                    hu_dst, hw_dst, n0 = hu_hi, hw_hi, i * 512 - HALF
                nc.sync.dma_start(
                    out=hu_dst[n0 : n0 + 512, :].rearrange("(p s) d -> p s d", p=P),
                    in_=ot3[:, :, 0:OUT_DIM],
                )
                nc.scalar.dma_start(
                    out=hw_dst[n0 : n0 + 512, :].rearrange("(p s) d -> p s d", p=P),
                    in_=ot3[:, :, OUT_DIM : 2 * OUT_DIM],
                )

            # ---- Phase 2 groups: per-edge messages ------------------------
            def p2_group(G):
                seg = 0
                while G >= SEG_EDGE_START[seg + 1] // G_EDGES:
                    seg += 1
                hu_src = (hu_lo if seg < 2 else hu_hi)[:]
                hw_src = (hw_lo if seg % 2 == 0 else hw_hi)[:]
                eb = epool.tile([P, G_EDGES // 2], BF16)
                nc.scalar.dma_start(out=eb[:], in_=eP[G])

                hu_g = gpool.tile([P, G_EDGES], BF16, tag="hu_g")
                nc.gpsimd.dma_gather(
                    out_ap=hu_g[:].rearrange("p (c d) -> p c d", c=TPG),
                    in_ap=hu_src,
                    idxs_ap=sidx_t[:, G * (G_EDGES // 16) : (G + 1) * (G_EDGES // 16)],
                    num_idxs=G_EDGES,
                    num_idxs_reg=G_EDGES,
                    elem_size=OUT_DIM,
                    single_packet=False,
                    queue_num=(2 * G) % 4,
                )
                hw_g = gpool.tile([P, G_EDGES], BF16, tag="hw_g")
                nc.gpsimd.dma_gather(
                    out_ap=hw_g[:].rearrange("p (c d) -> p c d", c=TPG),
                    in_ap=hw_src,
                    idxs_ap=tidx_t[:, G * (G_EDGES // 16) : (G + 1) * (G_EDGES // 16)],
                    num_idxs=G_EDGES,
                    num_idxs_reg=G_EDGES,
                    elem_size=OUT_DIM,
                    single_packet=False,
                    queue_num=(2 * G + 1) % 4,
                )

                mt = mpool.tile([P, G_EDGES], BF16)
                half_tiles = TPG // 2
                for hhalf in range(G_EDGES // 1024):
                    ps = p2psum.tile([P, 1024], F32)
                    for q in range(8):
                        t = hhalf * 8 + q
                        pb = 0 if t < half_tiles else EDGE_DIM
                        col = t % half_tiles
                        nc.tensor.matmul(
                            out=ps[:, q * P : (q + 1) * P],
                            lhsT=eb[pb : pb + EDGE_DIM, col * P : (col + 1) * P],
                            rhs=wet_t[pb : pb + EDGE_DIM, :],
                            start=True,
                            stop=True,
                        )
                    sl = slice(hhalf * 1024, (hhalf + 1) * 1024)
                    nc.vector.tensor_add(out=mt[:, sl], in0=ps[:], in1=hu_g[:, sl])
                    nc.vector.tensor_add(out=mt[:, sl], in0=mt[:, sl], in1=hw_g[:, sl])

                nc.sync.dma_start(
                    out=msgs[G * G_EDGES : (G + 1) * G_EDGES, :].rearrange(
                        "(p t) d -> p t d", p=P
                    ),
                    in_=mt[:].rearrange("p (t d) -> p t d", t=TPG),
                )

            # Emission order: lo table blocks first, then the (lo,lo) edge
            # segment (its gathers only need the lo tables), then the hi
            # blocks, then the remaining segments.
            for i in range(HALF // 512):
                p1_block(i)
            for G in range(CAPS[0]):
                p2_group(G)
            for i in range(HALF // 512, NODES_PAD // 512):
                p1_block(i)
            for G in range(CAPS[0], NGRP):
                p2_group(G)

    # Tile assigns each SWDGE DMA a DMASW completion-sem lane in *scheduled*
    # order, and a lane's sem may only ever be bumped from one SWDGE queue.
    # The scheduler reorders gathers, so a static queue rotation can put two
    # queues on one lane.  Rewrite queue_num = lane % 4 after scheduling.
    from concourse.tile_scheduler import PROC_NAME_TO_IDX

    sw0 = PROC_NAME_TO_IDX["DMASW0"]
    for inst in nc.inst_map.values():
        if isinstance(inst, mybir.InstDMAGatherAnt):
            lane = inst.bass_scheduled_proc - sw0
            assert 0 <= lane < 8, (inst.name, inst.bass_scheduled_proc)
            inst.queue_num = lane % 4

    nc.compile()
    return nc


def get_nc():
    if "nc" not in _CACHE:
        _CACHE["nc"] = _build()
    return _CACHE["nc"]


# tau: hT column j holds node tau(j), so that the phase-1 store
# "(p s) d -> p s d" (table row 4p+s <- psum partition p, slice s) lands
# node n at table row n.  psum partition p / slice s corresponds to hT
# column 512i + 128s + p, and the store writes it to table row 512i+4p+s.
def _tau(j):
    blk, off = j // 512, j % 512
    s, p = off // 128, off % 128
    return blk * 512 + 4 * p + s


_TAU = _tau(np.arange(NODES_PAD))


def _prep_in_maps(h, e, edge_index, W_e, W_hu, W_hw):
    """Returns (in_maps, pos_list): pos_list[c][i] = row of core c's device
    output holding original edge c*EPC+i."""
    h = np.ascontiguousarray(np.asarray(h, dtype=np.float32))
    e = np.ascontiguousarray(np.asarray(e, dtype=np.float32))
    src = np.asarray(edge_index[0]).astype(np.int64)
    tgt = np.asarray(edge_index[1]).astype(np.int64)

    hpad = np.zeros((NODES_PAD, IN_DIM), dtype=np.float32)
    hpad[:N_NODES] = h
    hT = np.ascontiguousarray(hpad[_TAU].T.astype(NPBF16))
    Wcat = np.ascontiguousarray(
        np.concatenate([np.asarray(W_hu, np.float32).T, np.asarray(W_hw, np.float32).T], axis=1)
    ).astype(NPBF16)
    # stacked twice so phase 2 has a copy at SBUF base partition 0 and 64
    WeT_np = np.asarray(W_e, np.float32).T
    WeT = np.ascontiguousarray(np.vstack([WeT_np, WeT_np])).astype(NPBF16)

    in_maps = []
    pos_list = []
    for c in range(NCORES):
        sl = slice(c * EPC, (c + 1) * EPC)
        sc, tc_, ec = src[sl], tgt[sl], e[sl]
        bucket = 2 * (sc >= HALF).astype(np.int64) + (tc_ >= HALF).astype(np.int64)

        e_pad = np.zeros((EPC_PAD, EDGE_DIM), dtype=np.float32)
        s16 = np.zeros((EPC_PAD,), dtype=np.int16)
        t16 = np.zeros((EPC_PAD,), dtype=np.int16)
        pos = np.empty((EPC,), dtype=np.int64)
        for b in range(4):
            selb = np.flatnonzero(bucket == b)
            if len(selb) > CAPS[b] * G_EDGES:
                raise RuntimeError(
                    f"bucket {b} overflow on core {c}: {len(selb)} > {CAPS[b] * G_EDGES}"
                )
            base = SEG_EDGE_START[b]
            pos[selb] = base + np.arange(len(selb))
            e_pad[base : base + len(selb)] = ec[selb]
            s16[base : base + len(selb)] = (sc[selb] - HALF * (b >> 1)).astype(np.int16)
            t16[base : base + len(selb)] = (tc_[selb] - HALF * (b & 1)).astype(np.int16)

        # device msgs row for padded slot j: group g = j // G_EDGES,
        # within-group slot sl sits at psum partition p = sl % 128, tile
        # t = sl // 128, stored at DRAM row g*G_EDGES + TPG*p + t.
        g_, sl_ = pos // G_EDGES, pos % G_EDGES
        pos = g_ * G_EDGES + TPG * (sl_ % P) + sl_ // P

        ePc = np.ascontiguousarray(
            e_pad.reshape(NGRP, 2, G_EDGES // 2, EDGE_DIM).transpose(0, 1, 3, 2)
        ).reshape(NGRP, P, G_EDGES // 2).astype(NPBF16)

        # dma_gather index layout: value j of group g sits at
        # [j % 16, g*(G/16) + j//16], replicated across the 8 gpsimd cores.
        def idx_layout(v16):
            a16 = v16.reshape(NGRP, G_EDGES // 16, 16).transpose(2, 0, 1).reshape(
                16, NGRP * (G_EDGES // 16)
            )
            return np.ascontiguousarray(np.tile(a16, (8, 1)))

        in_maps.append(
            {
                "hT": hT,
                "Wcat": Wcat,
                "WeT": WeT,
                "eP": ePc,
                "sidx": idx_layout(s16),
                "tidx": idx_layout(t16),
            }
        )
        pos_list.append(pos)
    return in_maps, pos_list


def _install_ntff_hook():
    """Best-effort: register the axon NTFF profile hook when the image's
    antenv package lacks axon_hooks (needed only for trace=True runs)."""
    import sys
    import types

    try:
        from antenv.axon_hooks import get_axon_ntff_profile_hook  # noqa: F401

        return
    except ImportError:
        pass
    try:
        from trn_agent_boot.trn_boot import _ntff_profile_via_ctypes

        hook = _ntff_profile_via_ctypes("/opt/axon/libaxon_pjrt.so")
        mod = types.ModuleType("antenv.axon_hooks")
        mod._hook = hook
        mod.get_axon_ntff_profile_hook = lambda: mod._hook
        mod.set_axon_ntff_profile_hook = lambda h: setattr(mod, "_hook", h)
        sys.modules["antenv.axon_hooks"] = mod
        import antenv

        antenv.axon_hooks = mod
    except Exception:
        pass


def kernel(h, e, edge_index, W_e, W_hu, W_hw):
    nc = get_nc()
    in_maps, pos_list = _prep_in_maps(h, e, edge_index, W_e, W_hu, W_hw)
    trace = bool(int(os.environ.get("KERNEL_TRACE", "0")))
    if trace:
        _install_ntff_hook()
    res = run_bass_kernel_spmd(nc, in_maps, list(range(NCORES)), trace=trace)
    LAST["exec_time_ns"] = res.exec_time_ns
    LAST["results"] = res
    out = np.empty((N_EDGES, OUT_DIM), dtype=np.float32)
    for c in range(NCORES):
        m = np.asarray(res.results[c]["msgs"])
        out[c * EPC : (c + 1) * EPC] = m[pos_list[c]].astype(np.float32)
    return out
